# revision 4
# baseline (speedup 1.0000x reference)
"""AttentionNetPooling on 8 Trainium2 NeuronCores.

Math (see reference): scores = MLP(z); weights = softmax(scores) over ALL
nodes; out[g] = sum_{i in g} weights[i] * z[i, :256] / count[g].

Strategy (single pass over z, data-parallel over graph-contiguous node
shards; softmax computed unnormalized (scores are O(1) with this data, and
softmax is shift-invariant), with the global denominator AllReduced):

Host: partition graphs into 8 contiguous ranges balanced by node count;
each core's range splits into 2 windows of <=128 graphs; each window's node
span is zero-padded to a common tile count (T0/T1 tiles of 128 nodes) so the
SPMD program is identical across cores. Pads are killed via an exp bias of
-1e5 and a one-hot column index of -1. Per-graph counts come from
np.bincount on batch_index (index metadata only).

Device, per 256-node chunk: PE-transpose z into feature-major (fp32r
pass-through), MLP h^T = W1T^T @ z^T (fp32r), relu+b1 on ACT, per-node
scores via lhsT=h matmuls, w = exp(s + b2) on ACT (pad-killing bias),
one-hot = (iota == colidx) * w on DVE, and a per-graph weighted-sum matmul
accumulating into a persistent PSUM bank per window. At the end the softmax
denominator is reduced across partitions (gpsimd) and cores (AllReduce),
and each window's PSUM block is scaled by 1/(S*count) and DMAed out.
"""
import numpy as np

import concourse.bass as bass
import concourse.bacc as bacc
import concourse.tile as tile
import concourse.mybir as mybir
import concourse.bass_isa as bass_isa
from concourse.bass_utils import run_bass_kernel_spmd

F32 = mybir.dt.float32
F32R = mybir.dt.float32r
AF = mybir.ActivationFunctionType
ALU = mybir.AluOpType

NCORES = 8
P = 128           # partitions / nodes per tile
IN_DIM = 320
POOL = 256
HID = 128
PAD_BIAS = -1.0e5  # exp(s + b2 + PAD_BIAS) == 0 for pad nodes

# test.py hooks: set trace=True to NTFF-profile; LAST_RESULT holds the
# BassKernelResults of the most recent kernel() call.
PROFILE = {"trace": False, "tmpdir": None}
LAST_RESULT = None

_BUILD_CACHE = {}


def _plan(batch_index, num_graphs):
    """Partition graphs into 8 node-balanced contiguous ranges, split each
    into 2 windows of <=128 graphs, pad window node spans to shared tile
    counts T0/T1 (each even, so tiles pair into 256-node chunks)."""
    G = int(num_graphs)
    N = batch_index.shape[0]
    counts = np.bincount(batch_index, minlength=G).astype(np.int64)
    cum = np.concatenate([[0], np.cumsum(counts)])  # cum[g] = first node of g

    # graph range per core, balanced by node count, capped at 256 graphs
    bounds = [0]
    for c in range(1, NCORES):
        g = int(np.searchsorted(cum, c * N / NCORES))
        g = max(bounds[-1], min(g, G))
        g = max(g, G - 256 * (NCORES - c))   # leave <=256 per remaining core
        g = min(g, bounds[-1] + 256)
        bounds.append(g)
    bounds.append(G)

    cores = []
    for c in range(NCORES):
        g_lo, g_hi = bounds[c], bounds[c + 1]
        assert g_hi - g_lo <= 256
        # split into 2 windows balanced by nodes, each <=128 graphs
        half = (cum[g_lo] + cum[g_hi]) / 2
        m = int(np.searchsorted(cum, half))
        m = max(g_lo, min(m, g_lo + 128))
        m = max(m, g_hi - 128)
        m = min(m, g_hi)
        wins = []
        for a, b in ((g_lo, m), (m, g_hi)):
            wins.append({"g_lo": a, "g_hi": b,
                         "n_lo": int(cum[a]), "n_hi": int(cum[b])})
        cores.append(wins)

    # tiles per window: even count of 128-node tiles covering the max span
    T = [2 * max(1, -(-max(cores[c][w]["n_hi"] - cores[c][w]["n_lo"]
                        for c in range(NCORES)) // 256)) for w in range(2)]
    return counts, cores, T


def _build_inputs(z, batch_index, W1, b1, W2, b2, counts, cores, T):
    nT = T[0] + T[1]
    Npad = nT * P
    b2s = float(np.asarray(b2).reshape(-1)[0])

    # shared constants
    W1T = np.zeros((P, 384), dtype=np.float32)  # [k-in-chunk, 128*chunk + h]
    w1t = np.ascontiguousarray(W1.T)            # [320, 128]
    for ch in range(3):
        k0, k1 = 128 * ch, min(128 * (ch + 1), IN_DIM)
        W1T[: k1 - k0, 128 * ch: 128 * ch + HID] = w1t[k0:k1]
    W2T = np.ascontiguousarray(W2.reshape(1, HID).T)       # [128, 1]
    b1c = np.asarray(b1, dtype=np.float32).reshape(HID, 1)
    ident = np.eye(P, dtype=np.float32)
    iota = np.tile(np.arange(P, dtype=np.float32), (P, 1))  # [128,128]

    in_maps = []
    for c in range(NCORES):
        zp = np.zeros((Npad, IN_DIM), dtype=np.float32)
        colidx = np.full(Npad, -1.0, dtype=np.float32)
        biascol = np.full(Npad, b2s + PAD_BIAS, dtype=np.float32)
        cnt = np.ones((P, 2), dtype=np.float32)
        for w in range(2):
            win = cores[c][w]
            base = T[0] * P if w else 0
            n = win["n_hi"] - win["n_lo"]
            zp[base: base + n] = z[win["n_lo"]: win["n_hi"]]
            colidx[base: base + n] = (
                batch_index[win["n_lo"]: win["n_hi"]] - win["g_lo"]
            ).astype(np.float32)
            biascol[base: base + n] = b2s
            ng = win["g_hi"] - win["g_lo"]
            cnt[:ng, w] = np.maximum(
                counts[win["g_lo"]: win["g_hi"]], 1).astype(np.float32)
        in_maps.append({
            "z": zp,
            "colidx": np.ascontiguousarray(colidx.reshape(nT, P).T),
            "biascol": np.ascontiguousarray(biascol.reshape(nT, P).T),
            "cnt": cnt,
            "w1t": W1T, "w2t": W2T, "b1": b1c,
            "ident": ident, "iota": iota,
        })
    return in_maps


def _build_program(T):
    key = tuple(T)
    if key in _BUILD_CACHE:
        return _BUILD_CACHE[key]
    nT = T[0] + T[1]
    Npad = nT * P

    nc = bacc.Bacc("TRN2", target_bir_lowering=False, debug=False,
                   num_devices=NCORES)
    z_d = nc.dram_tensor("z", [Npad, IN_DIM], F32, kind="ExternalInput").ap()
    colidx_d = nc.dram_tensor("colidx", [P, nT], F32, kind="ExternalInput").ap()
    biascol_d = nc.dram_tensor("biascol", [P, nT], F32, kind="ExternalInput").ap()
    cnt_d = nc.dram_tensor("cnt", [P, 2], F32, kind="ExternalInput").ap()
    w1t_d = nc.dram_tensor("w1t", [P, 384], F32, kind="ExternalInput").ap()
    w2t_d = nc.dram_tensor("w2t", [HID, 1], F32, kind="ExternalInput").ap()
    b1_d = nc.dram_tensor("b1", [HID, 1], F32, kind="ExternalInput").ap()
    ident_d = nc.dram_tensor("ident", [P, P], F32, kind="ExternalInput").ap()
    iota_d = nc.dram_tensor("iota", [P, P], F32, kind="ExternalInput").ap()
    out_d = nc.dram_tensor("out", [2 * P, POOL], F32, kind="ExternalOutput").ap()

    cc_in = nc.dram_tensor("cc_in", [P], F32)
    cc_out = nc.dram_tensor("cc_out", [P], F32, addr_space="Shared")

    # z as [chunk, p, (A/B), 320]
    z_r = z_d.rearrange("(q b p) d -> q p b d", b=2, p=P)

    with tile.TileContext(nc) as tc:
        with tc.tile_pool(name="const", bufs=1) as cpool, \
             tc.tile_pool(name="zin", bufs=4) as zpool, \
             tc.tile_pool(name="zt", bufs=3) as ztpool, \
             tc.tile_pool(name="hs", bufs=3) as hspool, \
             tc.tile_pool(name="oh", bufs=3) as ohpool, \
             tc.tile_pool(name="fin", bufs=1) as fpool, \
             tc.tile_pool(name="ps_a", bufs=2, space="PSUM") as psa, \
             tc.tile_pool(name="ps_b", bufs=2, space="PSUM") as psb, \
             tc.tile_pool(name="ps_h", bufs=2, space="PSUM") as psh, \
             tc.tile_pool(name="ps_B", bufs=1, space="PSUM") as psB:

            w1t_sb = cpool.tile([P, 384], F32R)
            nc.sync.dma_start(w1t_sb[:], w1t_d[:].bitcast(F32R))
            w2t_sb = cpool.tile([HID, 1], F32)
            nc.sync.dma_start(w2t_sb[:], w2t_d[:])
            b1_sb = cpool.tile([HID, 1], F32)
            nc.sync.dma_start(b1_sb[:], b1_d[:])
            ident_sb = cpool.tile([P, P], F32R)
            nc.sync.dma_start(ident_sb[:], ident_d[:].bitcast(F32R))
            iota_sb = cpool.tile([P, P], F32)
            nc.sync.dma_start(iota_sb[:], iota_d[:])
            colidx_sb = cpool.tile([P, nT], F32)
            nc.sync.dma_start(colidx_sb[:], colidx_d[:])
            bias_sb = cpool.tile([P, nT], F32)
            nc.sync.dma_start(bias_sb[:], biascol_d[:])
            cnt_sb = cpool.tile([P, 2], F32)
            nc.sync.dma_start(cnt_sb[:], cnt_d[:])
            w_all = cpool.tile([P, nT], F32)

            Bps0 = psB.tile([P, POOL], F32, tag="B0")
            Bps1 = psB.tile([P, POOL], F32, tag="B1")
            Bps = [Bps0, Bps1]

            for w in range(2):
                nch = T[w] // 2
                for q in range(nch):
                    gq = (T[0] // 2 if w else 0) + q     # global chunk idx
                    tA = (T[0] if w else 0) + 2 * q      # global tile idx
                    tB = tA + 1

                    z_sb = zpool.tile([P, 2, IN_DIM], F32R, tag="z")
                    nc.sync.dma_start(z_sb[:], z_r[gq].bitcast(F32R))
                    zA = z_sb[:, 0]
                    zB = z_sb[:, 1]

                    # transpose z into feature-major (pass-through, fp32r)
                    zt_a = psa.tile([P, 512], F32R, tag="zta")
                    nc.tensor.transpose(zt_a[:, 0:128], zA[:, 0:128], ident_sb[:])
                    nc.tensor.transpose(zt_a[:, 128:256], zB[:, 0:128], ident_sb[:])
                    nc.tensor.transpose(zt_a[:, 256:384], zA[:, 128:256], ident_sb[:])
                    nc.tensor.transpose(zt_a[:, 384:512], zB[:, 128:256], ident_sb[:])
                    zt_b = psb.tile([P, 258], F32, tag="ztb")
                    nc.tensor.transpose(zt_b[0:64, 0:128].bitcast(F32R),
                                        zA[:, 256:320], ident_sb[:])
                    nc.tensor.transpose(zt_b[0:64, 128:256].bitcast(F32R),
                                        zB[:, 256:320], ident_sb[:])

                    zt_sb = ztpool.tile([P, 768], F32R, tag="zt")
                    nc.scalar.copy(zt_sb[:, 0:512], zt_a[:])
                    nc.vector.tensor_copy(zt_sb[0:64, 512:768],
                                          zt_b[0:64, 0:256].bitcast(F32R))

                    # MLP layer 1 (contraction over features)
                    h_ps = psh.tile([P, 256], F32, tag="h")
                    nc.tensor.matmul(h_ps[:], w1t_sb[:, 0:128],
                                     zt_sb[:, 0:256], start=True, stop=False)
                    nc.tensor.matmul(h_ps[:], w1t_sb[:, 128:256],
                                     zt_sb[:, 256:512], start=False, stop=False)
                    nc.tensor.matmul(h_ps[:], w1t_sb[0:64, 256:384],
                                     zt_sb[0:64, 512:768], start=False, stop=True)
                    hs = hspool.tile([P, 256], F32, tag="hs")
                    nc.scalar.activation(hs[:], h_ps[:], AF.Relu, bias=b1_sb[:])

                    # scores: s[n] = hs[:, n] . W2
                    nc.tensor.matmul(zt_b[:, 256:257], hs[:, 0:128],
                                     w2t_sb[:], start=True, stop=True)
                    nc.tensor.matmul(zt_b[:, 257:258], hs[:, 128:256],
                                     w2t_sb[:], start=True, stop=True)

                    # w = exp(s + b2) (pad bias kills padding nodes)
                    for j, t in ((0, tA), (1, tB)):
                        nc.scalar.activation(
                            w_all[:, t: t + 1], zt_b[:, 256 + j: 257 + j],
                            AF.Exp, bias=bias_sb[:, t: t + 1])

                    # one-hot(graph) * w
                    oh = ohpool.tile([P, 256], F32R, tag="oh")
                    for j, t in ((0, tA), (1, tB)):
                        nc.vector.tensor_scalar(
                            oh[:, 128 * j: 128 * (j + 1)], iota_sb[:],
                            colidx_sb[:, t: t + 1], w_all[:, t: t + 1],
                            ALU.is_equal, ALU.mult)

                    # per-graph weighted sums
                    nc.tensor.matmul(Bps[w][:], oh[:, 0:128], zA[:, 0:POOL],
                                     start=(q == 0), stop=False)
                    nc.tensor.matmul(Bps[w][:], oh[:, 128:256], zB[:, 0:POOL],
                                     start=False, stop=(q == nch - 1))

            # softmax denominator: S = sum over all cores/nodes of w
            wsum = fpool.tile([P, 1], F32, tag="wsum")
            nc.vector.tensor_reduce(wsum[:], w_all[:], mybir.AxisListType.X,
                                    ALU.add)
            s_rep = fpool.tile([P, 1], F32, tag="srep")
            nc.gpsimd.partition_all_reduce(s_rep[:], wsum[:], P,
                                           bass_isa.ReduceOp.add)
            nc.sync.dma_start(cc_in.ap()[:], s_rep[:, 0])
            nc.gpsimd.collective_compute(
                "AllReduce", ALU.add, ins=[cc_in.ap()[:]],
                outs=[cc_out.ap()[:]],
                replica_groups=[list(range(NCORES))])
            s_glob = fpool.tile([P, 1], F32, tag="sglob")
            nc.sync.dma_start(s_glob[:, 0], cc_out.ap()[:])

            # out[g] = B[g] / (S * count[g])
            for w in range(2):
                denom = fpool.tile([P, 1], F32, tag=f"den{w}")
                nc.vector.tensor_tensor(denom[:], cnt_sb[:, w: w + 1],
                                        s_glob[:], ALU.mult)
                rec = fpool.tile([P, 1], F32, tag=f"rec{w}")
                nc.vector.reciprocal(rec[:], denom[:])
                outw = fpool.tile([P, POOL], F32, tag=f"out{w}")
                nc.vector.tensor_scalar(outw[:], Bps[w][:], rec[:], None,
                                        ALU.mult)
                nc.sync.dma_start(out_d[P * w: P * (w + 1), :], outw[:])

    nc.compile()
    _BUILD_CACHE[key] = nc
    return nc


def kernel(z, batch_index, W1, b1, W2, b2, num_graphs):
    global LAST_RESULT
    z = np.asarray(z, dtype=np.float32)
    batch_index = np.asarray(batch_index)
    G = int(num_graphs)

    counts, cores, T = _plan(batch_index, G)
    in_maps = _build_inputs(z, batch_index, np.asarray(W1), np.asarray(b1),
                            np.asarray(W2), np.asarray(b2), counts, cores, T)
    nc = _build_program(T)

    res = run_bass_kernel_spmd(
        nc, in_maps, list(range(NCORES)),
        trace=PROFILE["trace"],
        **({"tmpdir": PROFILE["tmpdir"]} if PROFILE["tmpdir"] else {}))
    LAST_RESULT = res

    out = np.zeros((G, POOL), dtype=np.float32)
    for c in range(NCORES):
        for w in range(2):
            win = cores[c][w]
            ng = win["g_hi"] - win["g_lo"]
            if ng:
                out[win["g_lo"]: win["g_hi"]] = \
                    res.results[c]["out"][P * w: P * w + ng]
    return out
